# revision 7
# baseline (speedup 1.0000x reference)
"""KVGather kernel for Trainium2 (8 NeuronCores, SPMD data-parallel over batch).

Problem: kv (16, 64, 196, 128) f32; r_idx/r_weight (16, 64, 4).
out[n, p, t] = r_weight[n, p, t] * kv[n, r_idx[n, p, t]]  -> (16, 64, 4, 196, 128)

Strategy (per core: 2 batches). The kernel is HBM-bandwidth bound, so both
sides of the traffic run in bf16 (tolerance is 2e-2; bf16 end-to-end keeps
max rel err ~6e-3):
  - kv is loaded once per batch as a single bf16 tensor [128, 12544]
    (partition h*64 + r = half h of region r, flat over (w2, c_kv)).
  - Gather runs on the PE array as a one-hot matmul: psum[m, :] =
    sel_mh.T @ kv_chunk, with sel a host-built {0,1} bf16 selection matrix.
  - PSUM eviction fuses the r_weight multiply (per-partition f32 scalar)
    and the f32->bf16 downconvert, alternating DVE/ACT 5:3 to match their
    throughput ratio.
  - Output is written bf16: one 6.4 MB DMA per (batch, group of 128
    gathers), 50 KB contiguous per partition; host upconverts to f32.

Per-core HBM traffic: 6.4 MB in + 25.7 MB out = 32.1 MB (vs 51.4+19.3 for
the f32-out/3-term baseline), against a ~358 GB/s per-core HBM limit.

Everything is static: one compiled program for all cores and all inputs;
indices/weights only enter through input tensors (sel, wt).
"""

import sys

if "/opt/trn_rl_repo" not in sys.path:
    sys.path.insert(0, "/opt/trn_rl_repo")

import numpy as np
import ml_dtypes

import concourse.bass as bass
import concourse.bacc as bacc
import concourse.mybir as mybir
from concourse import tile
from concourse.bass_utils import run_bass_kernel_spmd

BF16 = ml_dtypes.bfloat16

# Problem constants
N, P2, TOPK, W2, C_KV = 16, 64, 4, 196, 128
REG = W2 * C_KV  # 25088 elems per region
RHALF = REG // 2  # 12544 per region half
N_CORES = 8
B = N // N_CORES  # batches per core = 2
G = P2 * TOPK  # gathers per batch = 256
MG = G // 128  # m-groups of 128 gathers = 2
CH = 1792  # psum chunk: 3.5 banks of f32, 7 equal chunks per half
NCH = RHALF // CH  # 7
MM = 512  # max moving free dim per matmul

_COMPILED = None
RUN_KWARGS = {}  # test harness may set e.g. {"trace": True}
LAST_RESULTS = None  # BassKernelResults of the last run (for profiling)


def _build():
    nc = bacc.Bacc("TRN2", target_bir_lowering=False, debug=False, num_devices=N_CORES)
    f32, bf16 = mybir.dt.float32, mybir.dt.bfloat16

    kv_d = nc.dram_tensor("kvb", [B, 128, RHALF], bf16, kind="ExternalInput").ap()
    sel_d = nc.dram_tensor("sel", [128, B * MG * 2 * 128], bf16, kind="ExternalInput").ap()
    wt_d = nc.dram_tensor("wt", [128, B * MG], f32, kind="ExternalInput").ap()
    out_d = nc.dram_tensor("out", [B, G, REG], bf16, kind="ExternalOutput").ap()

    with tile.TileContext(nc) as tc:
        with (
            tc.tile_pool(name="rhs", bufs=2) as rhs_pool,
            tc.tile_pool(name="const", bufs=1) as const_pool,
            tc.tile_pool(name="psum", bufs=2, space="PSUM") as psum_pool,
            tc.tile_pool(name="outp", bufs=2) as out_pool,
        ):
            sel_sb = const_pool.tile([128, B * MG * 2 * 128], bf16)
            wt_sb = const_pool.tile([128, B * MG], f32)
            nc.sync.dma_start(sel_sb[:], sel_d)
            nc.sync.dma_start(wt_sb[:], wt_d)

            # Load all kv upfront on the SWDGE (gpsimd) path, leaving both
            # HWDGE rings free for output; stripes are chunk-aligned so the
            # first matmuls only wait on stripe 0.
            stripes = [(0, 1792), (1792, 3584), (3584, 7168), (7168, 10752), (10752, RHALF)]
            kv_sbs = []
            for b in range(B):
                kv_sb = rhs_pool.tile([128, RHALF], bf16, tag="term", name=f"kv{b}")
                kv_sbs.append(kv_sb)
                for s0, s1 in stripes:
                    nc.gpsimd.dma_start(kv_sb[:, s0:s1], kv_d[b][:, s0:s1])

            ev = 0
            for b in range(B):
                kv_sb = kv_sbs[b]
                for mg in range(MG):
                    wcol = wt_sb[:, b * MG + mg : b * MG + mg + 1]
                    ot = out_pool.tile([128, REG], bf16, tag="ot")
                    for h in range(2):
                        si = (b * MG + mg) * 2 + h
                        sel_ap = sel_sb[:, si * 128 : (si + 1) * 128]
                        for c in range(NCH):
                            ps = psum_pool.tile([128, CH], f32, tag="ps")
                            for m0 in range(0, CH, MM):
                                mw = min(MM, CH - m0)
                                col = c * CH + m0
                                nc.tensor.matmul(
                                    ps[:, m0 : m0 + mw],
                                    sel_ap,
                                    kv_sb[:, col : col + mw],
                                )
                            dst = ot[:, h * RHALF + c * CH : h * RHALF + (c + 1) * CH]
                            # alternate DVE/ACT 1:1 (both ~110 G elem/s on
                            # f32 PSUM reads)
                            if ev % 2 == 0:
                                nc.vector.tensor_scalar_mul(dst, ps[:], wcol)
                            else:
                                nc.scalar.activation(
                                    dst,
                                    ps[:],
                                    mybir.ActivationFunctionType.Copy,
                                    scale=wcol,
                                )
                            ev += 1
                        # one 3.2 MB output DMA per half, ping-ponging between
                        # the two HWDGE rings (sync / scalar): while one ring
                        # drains, the other's next transfer is already queued,
                        # so big transfers run back-to-back with no issue gap
                        hid = ((b * MG + mg) * 2 + h) % 2
                        ring = nc.sync if hid == 0 else nc.scalar
                        ring.dma_start(
                            out_d[
                                b,
                                mg * 128 : (mg + 1) * 128,
                                h * RHALF : (h + 1) * RHALF,
                            ],
                            ot[:, h * RHALF : (h + 1) * RHALF],
                        )

    nc.compile()
    return nc


def _get_nc():
    global _COMPILED
    if _COMPILED is None:
        _COMPILED = _build()
    return _COMPILED


def _prep_core(kv_c: np.ndarray, idx_c: np.ndarray, w_c: np.ndarray) -> dict:
    """kv_c (B, 64, 196, 128) f32, idx_c (B, 64, 4) int, w_c (B, 64, 4) f32."""
    # rhs layout [B, 128, RHALF]: partition h*64 + r = half h of region r (flat)
    kvb = (
        kv_c.reshape(B, P2, 2, RHALF).transpose(0, 2, 1, 3).reshape(B, 128, RHALF)
    ).astype(BF16)

    idx_f = idx_c.reshape(B, G).astype(np.int64)
    w_f = w_c.reshape(B, G).astype(np.float32)

    sel = np.zeros((128, B, MG, 2, 128), dtype=BF16)
    k = np.arange(128)[:, None]
    for b in range(B):
        for mg in range(MG):
            im = idx_f[b, mg * 128 : (mg + 1) * 128][None, :]
            sel[:, b, mg, 0] = (k == im).astype(BF16)
            sel[:, b, mg, 1] = (k == im + 64).astype(BF16)
    sel = sel.reshape(128, B * MG * 2 * 128)

    wt = np.zeros((128, B * MG), dtype=np.float32)
    for b in range(B):
        for mg in range(MG):
            wt[:, b * MG + mg] = w_f[b, mg * 128 : (mg + 1) * 128]

    return {"kvb": kvb, "sel": sel, "wt": wt}


def kernel(r_idx: np.ndarray, r_weight: np.ndarray, kv: np.ndarray) -> np.ndarray:
    global LAST_RESULTS
    nc = _get_nc()
    kv = np.asarray(kv, dtype=np.float32)
    r_idx = np.asarray(r_idx)
    r_weight = np.asarray(r_weight, dtype=np.float32)

    in_maps = [
        _prep_core(
            kv[c * B : (c + 1) * B],
            r_idx[c * B : (c + 1) * B],
            r_weight[c * B : (c + 1) * B],
        )
        for c in range(N_CORES)
    ]

    res = run_bass_kernel_spmd(nc, in_maps, core_ids=list(range(N_CORES)), **RUN_KWARGS)
    LAST_RESULTS = res

    out = np.empty((N, P2, TOPK, W2, C_KV), dtype=np.float32)
    for c in range(N_CORES):
        o = res.results[c]["out"]  # (B, G, REG) bf16
        out[c * B : (c + 1) * B] = o.astype(np.float32).reshape(B, P2, TOPK, W2, C_KV)
    return out
